# revision 46
# baseline (speedup 1.0000x reference)
"""Trainium2 Bass kernel for nn_Block_56126632624726 (dense transformer block).

Reference computation (fp32, B=4, L=2048, D=1024, H=8 heads, hd=128):
    h = LayerNorm(x) * gamma + beta
    [q,k,v,lin,pre] = h @ w_qkv.T            (5*D outputs)
    attn = causal p-softmax attention (p=2)
    branch = [lin * gelu(pre), attn]
    out = x + branch @ w_out.T

Sharding: 8 cores = 4 batches (data parallel) x 2 tensor-parallel halves.
Within a batch pair, core j in {0,1} owns heads 4j..4j+3 (512 cols of each
of q/k/v) plus lin/pre cols 512j..512j+512, and the matching w_out input
columns. Each core emits a partial [2048, 1024] output; the host sums the
two partials per batch and adds the residual x (so no device collectives).

Per-core kernel, bf16 data path (fp32 PSUM accumulation):
  - gamma is folded into the weights host-side; beta (zero in practice)
    rides the projection evacuations / gelu bias when nonzero.
  - LayerNorm stats via PE ones-matmuls on x and x^2 (x^2 on ScalarE);
    rsqrt(var+eps) as exp(-0.5*ln(var+eps)) on ScalarE (no slow DVE
    reciprocal); per-token rows broadcast across partitions via a DRAM
    bounce.
  - Projections with weights resident in SBUF: QT/KT/gT dim-major
    (weight tiles stationary, 4 token-quarter matmuls per tile), V
    token-major (h tiles stationary, wv moving).
  - Attention per (head, quarter) with transposed scores ST[k,q]:
    f = exp(s*scale) (bf16, no max subtraction), causal mask template,
    e = f^2 on DVE, r[q] = sum_k e via ones-matmuls, OT = V.T @ f in
    PSUM.  Normalization batched at the end: all 16 r-rows -> one
    ln/exp rsqrt -> DRAM-bounce broadcast -> in-place scale of attnT.
  - Out-projection token-major: branch tiles stationary, w_out columns
    moving; writes out[L, D] fp32 directly.
"""

import numpy as np

# ---------------------------------------------------------------------------
# constants (hardcoded problem shapes)
# ---------------------------------------------------------------------------
B = 4
L = 2048
D = 1024
H = 8  # global heads
HD = 128
HL = 4  # heads per core
P = 128
KC = D // P  # 8 dim-chunks
NQ = 4  # token quarters
TQ = L // NQ  # 512
NT = L // P  # 16 token tiles
SCALE = float(HD) ** -0.5
EPS = 1e-5

_CACHED = {}


def _install_tile_drain_patch(tile, mybir):
    """walrus limits sem waits per SP CTRL instruction to 1; split the
    TileContext final drain's waits across sequential drain instructions."""
    from concourse.vector_clock import ScopedClock

    if getattr(tile.TileContext, "_drain_patched", False):
        return

    def _patched(self, tick_clock, wait_clock):
        drain_inst = self.nc.sync.drain()
        wait_clock.add_sem_waits(
            drain_inst.ins, ScopedClock({None: tick_clock.global_clock})
        )
        si = drain_inst.ins.sync_info
        waits = list(si.on_wait or []) if si else []
        if len(waits) > 1:
            si.on_wait = waits[:1]
            for w in waits[1:]:
                d2 = self.nc.sync.drain()
                d2.ins.sync_info = mybir.SyncInfo(on_wait=[w], on_update=[])
        self.nc.all_engine_barrier()
        popped = self.nc._tile_sem_poison_stack.pop()
        assert popped is self._sem_poison
        self.nc.clear_and_free_semaphores(list(self.sems.allocated().values()))
        self.nc.all_engine_barrier()

    tile.TileContext._drain_and_barrier = _patched
    tile.TileContext._drain_patched = True


def _split_multi_waits(nc, mybir):
    """This walrus build supports at most ONE sync-wait per instruction
    (single wait slot in every engine's 64B encoding). Tile's wait assignment
    can attach several. Engine streams execute in order (including SP's DMA
    triggers), so move extra waits onto same-engine nops inserted before the
    instruction."""
    eng_builder = {
        mybir.EngineType.PE: nc.tensor,
        mybir.EngineType.DVE: nc.vector,
        mybir.EngineType.Activation: nc.scalar,
        mybir.EngineType.SP: nc.sync,
        mybir.EngineType.Pool: nc.gpsimd,
    }

    def make_nop(engine):
        bi = eng_builder[engine].nop(nofuse=True)
        inst = bi.ins
        nc.cur_bb.bb.instructions.remove(inst)
        return inst

    for f in nc.m.functions:
        for bb in f.blocks:
            insts = bb.instructions
            rebuilt = []
            changed = False
            for inst in list(insts):
                si = inst.sync_info
                waits = list(si.on_wait or []) if si else []
                if len(waits) > 1:
                    changed = True
                    for w in waits[:-1]:
                        nop = make_nop(inst.engine)
                        nop.sync_info = mybir.SyncInfo(on_wait=[w], on_update=[])
                        rebuilt.append(nop)
                    si.on_wait = waits[-1:]
                rebuilt.append(inst)
            if changed:
                insts.clear()
                insts.extend(rebuilt)


def _build_nc(with_beta=False):
    import concourse.bass as bass
    import concourse.tile as tile
    from concourse import mybir

    _install_tile_drain_patch(tile, mybir)

    f32 = mybir.dt.float32
    bf = mybir.dt.bfloat16
    AF = mybir.ActivationFunctionType
    OP = mybir.AluOpType

    nc = bass.Bass()

    xT = nc.declare_dram_parameter("xT", [D, L], bf, isOutput=False)
    wq = nc.declare_dram_parameter("wqT", [D, 512], bf, isOutput=False)
    wk = nc.declare_dram_parameter("wkT", [D, 512], bf, isOutput=False)
    wv = nc.declare_dram_parameter("wvT", [D, 512], bf, isOutput=False)
    wl = nc.declare_dram_parameter("wlinT", [D, 512], bf, isOutput=False)
    wp = nc.declare_dram_parameter("wpreT", [D, 512], bf, isOutput=False)
    wo = nc.declare_dram_parameter("woT", [D, D], bf, isOutput=False)
    maskT = nc.declare_dram_parameter("maskT", [P, 896], bf, isOutput=False)
    # beta corrections (dim-major cols); zeros when not with_beta
    cqkl = nc.declare_dram_parameter("cqkl", [P, 3 * HL], f32, isOutput=False)
    cpre = nc.declare_dram_parameter("cpre", [P, HL], f32, isOutput=False)
    cvrow = nc.declare_dram_parameter("cvrow", [1, 512], bf, isOutput=False)
    out = nc.declare_dram_parameter("out", [L, D], bf, isOutput=True)

    xT_r = xT.rearrange("(o p) t -> p o t", p=P)  # [128, 8, 2048]
    wq_r = wq.rearrange("(o p) f -> p o f", p=P)  # [128, 8, 512]
    wk_r = wk.rearrange("(o p) f -> p o f", p=P)
    wv_r = wv.rearrange("(o p) f -> p o f", p=P)
    wl_r = wl.rearrange("(o p) f -> p o f", p=P)
    wp_r = wp.rearrange("(o p) f -> p o f", p=P)
    wo_r = wo.rearrange("(o p) f -> p o f", p=P)  # [128, 8, 1024]

    import concourse.bass as _b

    with tile.TileContext(nc) as tc:
        with tc.tile_pool(name="persist", bufs=1) as persist:
            masks = persist.tile([P, 896], bf)
            ones = persist.tile([P, 1], bf)
            nc.vector.memset(ones, 1.0)
            ones_row = persist.tile([P, P], bf)
            nc.vector.memset(ones_row, 1.0)
            epst = persist.tile([P, 1], f32)
            nc.vector.memset(epst, EPS)
            h = persist.tile([P, KC, L], bf)
            QT = persist.tile([P, HL, L], bf)
            KT = persist.tile([P, HL, L], bf)
            V = persist.tile([P, NT, 512], bf)  # token-major V
            gT = persist.tile([P, HL, L], bf)

            # x quarters first (they gate the LN stats matmuls; the weight
            # loads are not needed until phase B and would otherwise delay
            # the first PE work by tens of us)
            xs_pool = tc.alloc_tile_pool(name="xs_pool", bufs=1)
            xs = xs_pool.tile([P, KC, L], bf)
            for q in range(NQ):
                tsl = slice(TQ * q, TQ * q + TQ)
                if q == 0:
                    nc.sync.dma_start(out=xs[:, 0:4, tsl], in_=xT_r[:, 0:4, tsl])
                    nc.sync.dma_start(out=xs[:, 4:8, tsl], in_=xT_r[:, 4:8, tsl])
                else:
                    nc.sync.dma_start(out=xs[:, :, tsl], in_=xT_r[:, :, tsl])
            nc.sync.dma_start(out=masks[:], in_=maskT[:])

            # weights resident (w_qkv slices; wo loaded in phase C)
            wqs = persist.tile([P, KC, 512], bf)
            nc.sync.dma_start(out=wqs[:], in_=wq_r[:])
            wks = persist.tile([P, KC, 512], bf)
            nc.sync.dma_start(out=wks[:], in_=wk_r[:])
            wvs = persist.tile([P, KC, 512], bf)
            nc.sync.dma_start(out=wvs[:], in_=wv_r[:])
            wls = persist.tile([P, KC, 512], bf)
            nc.sync.dma_start(out=wls[:], in_=wl_r[:])
            wps = persist.tile([P, KC, 512], bf)
            nc.sync.dma_start(out=wps[:], in_=wp_r[:])
            if with_beta:
                cqk = persist.tile([P, 3 * HL], f32)
                nc.sync.dma_start(out=cqk, in_=cqkl[:])
                cpr = persist.tile([P, HL], f32)
                nc.sync.dma_start(out=cpr, in_=cpre[:])

            # ---------------- Phase A: LayerNorm ----------------
            with (
                tc.tile_pool(name="x2p", bufs=6) as x2p,
                tc.tile_pool(name="rows", bufs=2) as rows,
                tc.tile_pool(name="bc1", bufs=2) as bc1,
                tc.tile_pool(name="sps", bufs=1, space="PSUM") as sps,
            ):
                # LN stats: each quarter's reduction chain gets its OWN PSUM
                # bank, with the output row placed at partition 32q so the
                # four M=1 matmul chains run concurrently on distinct PE
                # column groups.  (Chains must not share a bank: start=True
                # clears the whole bank and races with sibling strips.)
                s1l, s2l = [], []
                for q in range(NQ):
                    s1l.append(sps.tile([97, TQ], f32, tag="s1", name=f"s1_{q}", bufs=4))
                    s2l.append(sps.tile([97, TQ], f32, tag="s2", name=f"s2_{q}", bufs=4))
                for k in range(KC):
                    for q in range(NQ):
                        tsl = slice(TQ * q, TQ * q + TQ)
                        nc.tensor.matmul(
                            s1l[q][32 * q : 32 * q + 1, :], lhsT=ones,
                            rhs=xs[:, k, tsl],
                            start=(k == 0), stop=(k == KC - 1),
                            tile_position=(0, 32 * q),
                        )
                for k in range(KC):
                    for q in range(NQ):
                        tsl = slice(TQ * q, TQ * q + TQ)
                        x2 = x2p.tile([P, TQ], bf, tag="x2")
                        if q % 2 == 0:
                            nc.scalar.activation(
                                out=x2[:], in_=xs[:, k, tsl], func=AF.Square
                            )
                        else:
                            nc.vector.tensor_mul(
                                out=x2[:], in0=xs[:, k, tsl], in1=xs[:, k, tsl]
                            )
                        nc.tensor.matmul(
                            s2l[q][32 * q : 32 * q + 1, :], lhsT=ones, rhs=x2[:],
                            start=(k == 0), stop=(k == KC - 1),
                            tile_position=(0, 32 * q),
                        )
                # row math on DVE at lane 32q (keeps ScalarE free for the
                # Ln/Exp batches); var and ln(var+eps) in place in the s2
                # bank.  Broadcasts across partitions via rank-1 PE matmuls;
                # the broadcast outputs reuse the (dead) stats banks via tag
                # aliasing to stay within the 8 PSUM banks.
                mbl, invbl = [], []
                mubl = []
                for q in range(NQ):
                    sq = slice(32 * q, 32 * q + 1)
                    mu = rows.tile([97, TQ], f32, tag="mu", bufs=2)
                    nc.vector.tensor_scalar_mul(out=mu[sq, :], in0=s1l[q][sq, :], scalar1=1.0 / D)
                    mu2 = rows.tile([97, TQ], f32, tag="mu2")
                    nc.vector.tensor_mul(out=mu2[sq, :], in0=mu[sq, :], in1=mu[sq, :])
                    nc.vector.tensor_scalar_mul(out=s2l[q][sq, :], in0=s2l[q][sq, :], scalar1=1.0 / D)
                    nc.vector.tensor_tensor(
                        out=s2l[q][sq, :], in0=s2l[q][sq, :], in1=mu2[sq, :], op=OP.subtract
                    )
                    nc.scalar.activation(
                        out=s2l[q][sq, :], in_=s2l[q][sq, :], func=AF.Ln, bias=epst[sq, :]
                    )
                    mub16 = rows.tile([97, TQ], bf, tag="mub16", bufs=4)
                    nc.vector.tensor_copy(out=mub16[sq, :], in_=mu[sq, :])
                    mubl.append(mub16)
                for q in range(NQ):
                    sq = slice(32 * q, 32 * q + 1)
                    bps = sps.tile([P, TQ], f32, tag="s1", name=f"mbps_{q}", bufs=4)
                    nc.tensor.matmul(
                        bps, lhsT=ones_row[sq, :], rhs=mubl[q][sq, :],
                        start=True, stop=True, tile_position=(32 * q, 0),
                    )
                    mb = bc1.tile([P, TQ], bf, tag="mb", bufs=4)
                    nc.vector.tensor_copy(out=mb[:], in_=bps)
                    mbl.append(mb)
                for q in range(NQ):
                    sq = slice(32 * q, 32 * q + 1)
                    invb16 = rows.tile([97, TQ], bf, tag="invb16", bufs=4)
                    nc.scalar.activation(
                        out=invb16[sq, :], in_=s2l[q][sq, :], func=AF.Exp, scale=-0.5
                    )
                    bps = sps.tile([P, TQ], f32, tag="s2", name=f"ibps_{q}", bufs=4)
                    nc.tensor.matmul(
                        bps, lhsT=ones_row[sq, :], rhs=invb16[sq, :],
                        start=True, stop=True, tile_position=(32 * q, 0),
                    )
                    invb = bc1.tile([P, TQ], bf, tag="invb", bufs=4)
                    nc.vector.tensor_copy(out=invb[:], in_=bps)
                    invbl.append(invb)
                # normalize chunk-outer so phase B's first weight groups (which
                # touch all four quarters of chunk k) unblock early
                for k in range(KC):
                    for q in range(NQ):
                        tsl = slice(TQ * q, TQ * q + TQ)
                        nc.vector.tensor_tensor(
                            out=h[:, k, tsl], in0=xs[:, k, tsl], in1=mbl[q][:],
                            op=OP.subtract,
                        )
                        nc.vector.tensor_mul(
                            out=h[:, k, tsl], in0=h[:, k, tsl], in1=invbl[q][:]
                        )

            xs_pool.release()

            # ---------------- Phase B: projections ----------------
            with (
                tc.tile_pool(name="gel", bufs=4) as gelp,
                tc.tile_pool(name="pps", bufs=8, space="PSUM") as pps,
            ):
                # QT / KT (dim-major): weight tile stationary over 4 quarters
                for di, (dst, ws) in enumerate(((QT, wqs), (KT, wks))):
                    for m in range(HL):
                        pss = [
                            pps.tile([P, TQ], f32, tag="mm", name=f"qk{di}_{m}_{i}")
                            for i in range(NQ)
                        ]
                        for k in range(KC):
                            for qt in range(NQ):
                                nc.tensor.matmul(
                                    pss[qt],
                                    lhsT=ws[:, k, P * m : P * m + P],
                                    rhs=h[:, k, TQ * qt : TQ * qt + TQ],
                                    start=(k == 0), stop=(k == KC - 1),
                                )
                        for qt in range(NQ):
                            osl = (slice(None), m, slice(TQ * qt, TQ * qt + TQ))
                            if with_beta:
                                nc.vector.tensor_scalar(
                                    out=dst[osl], in0=pss[qt],
                                    scalar1=cqk[:, HL * di + m : HL * di + m + 1],
                                    scalar2=None, op0=OP.add,
                                )
                            else:
                                nc.vector.tensor_copy(out=dst[osl], in_=pss[qt])

                # gT = lin * gelu(pre) (dim-major)
                for m in range(HL):
                    psp = [
                        pps.tile([P, TQ], f32, tag="mm", name=f"pp_{m}_{i}")
                        for i in range(NQ)
                    ]
                    for k in range(KC):
                        for qt in range(NQ):
                            nc.tensor.matmul(
                                psp[qt],
                                lhsT=wps[:, k, P * m : P * m + P],
                                rhs=h[:, k, TQ * qt : TQ * qt + TQ],
                                start=(k == 0), stop=(k == KC - 1),
                            )
                    gels = []
                    for qt in range(NQ):
                        gel = gelp.tile([P, TQ], bf, tag="gel")
                        if with_beta:
                            nc.scalar.activation(
                                out=gel[:], in_=psp[qt], func=AF.Gelu,
                                bias=cpr[:, m : m + 1],
                            )
                        else:
                            nc.scalar.activation(out=gel[:], in_=psp[qt], func=AF.Gelu)
                        gels.append(gel)
                    psl = [
                        pps.tile([P, TQ], f32, tag="mm", name=f"pl_{m}_{i}")
                        for i in range(NQ)
                    ]
                    for k in range(KC):
                        for qt in range(NQ):
                            nc.tensor.matmul(
                                psl[qt],
                                lhsT=wls[:, k, P * m : P * m + P],
                                rhs=h[:, k, TQ * qt : TQ * qt + TQ],
                                start=(k == 0), stop=(k == KC - 1),
                            )
                    for qt in range(NQ):
                        osl = (slice(None), m, slice(TQ * qt, TQ * qt + TQ))
                        if with_beta:
                            lsb = gelp.tile([P, TQ], f32, tag="lsb")
                            nc.vector.tensor_scalar(
                                out=lsb[:], in0=psl[qt],
                                scalar1=cqk[:, 2 * HL + m : 2 * HL + m + 1],
                                scalar2=None, op0=OP.add,
                            )
                            nc.vector.tensor_mul(out=gT[osl], in0=lsb[:], in1=gels[qt])
                        else:
                            nc.vector.tensor_mul(out=gT[osl], in0=psl[qt], in1=gels[qt])

                # V (token-major): h tiles stationary, wv moving
                if with_beta:
                    cvb = persist.tile([P, 512], bf)
                    rdv = rowd_v = tc.alloc_tile_pool(name="rdv", bufs=1, space="DRAM")
                    rv = rdv.tile([1, 512], bf)
                    nc.sync.dma_start(out=rv, in_=cvrow[:])
                    bapv = _b.AP(tensor=rv.tensor, offset=rv.offset, ap=[[0, P], [1, 512]])
                    nc.sync.dma_start(out=cvb[:], in_=bapv)
                for i in range(NT):
                    ps = pps.tile([P, 512], f32, tag="mm", name=f"v_{i}")
                    for k in range(KC):
                        nc.tensor.matmul(
                            ps,
                            lhsT=h[:, k, P * i : P * i + P],
                            rhs=wvs[:, k, :],
                            start=(k == 0), stop=(k == KC - 1),
                        )
                    if with_beta:
                        nc.vector.tensor_add(out=V[:, i, :], in0=ps, in1=cvb[:])
                    else:
                        nc.vector.tensor_copy(out=V[:, i, :], in_=ps)
                if with_beta:
                    rowd_v.release()

            # ---------------- Phase C: attention ----------------
            attn_pool = tc.alloc_tile_pool(name="attn_pool", bufs=1)
            attnT = attn_pool.tile([P, HL, L], bf)
            rrows = attn_pool.tile([16, TQ], f32)
            rrows2 = attn_pool.tile([16, TQ], f32)
            wop = tc.alloc_tile_pool(name="wop", bufs=1)
            wos = wop.tile([P, KC, D], bf)
            nc.sync.dma_start(out=wos[:], in_=wo_r[:])

            with (
                tc.tile_pool(name="fp", bufs=3) as fp,
                tc.tile_pool(name="ep", bufs=6) as ep,
                tc.tile_pool(name="cb", bufs=4) as cbp,
                tc.tile_pool(name="crd", bufs=1, space="DRAM") as crd,
                tc.tile_pool(name="stps", bufs=3, space="PSUM") as stps,
                tc.tile_pool(name="ops", bufs=2, space="PSUM") as ops,
                tc.tile_pool(name="arps", bufs=2, space="PSUM") as arps,
            ):
                # zero rrows2 up front: the J=0 slots keep the zeros (those
                # (h,J) use a single r chain; see below), the rest are
                # overwritten by their row DMAs
                nc.vector.memset(rrows2[:], 0.0)
                for hh in range(HL):
                    for J in range(NQ):
                        jsl = slice(TQ * J, TQ * J + TQ)
                        nt = 4 * J + 4
                        o_ps = ops.tile([P, TQ], f32, tag="o")
                        # even/odd-t partial sums of e in two separate PSUM
                        # banks, output rows at partitions 0/32 so the two
                        # M=1 matmul chains overlap on distinct PE column
                        # groups (chains must not share a bank: start=True
                        # clears the whole bank).  J=0 uses a single chain:
                        # its t=1 tile is partial-width and a partial-width
                        # start would leave the rest of the bank unwritten.
                        r_a = arps.tile([1, TQ], f32, tag="ra", bufs=1)
                        if J > 0:
                            r_b = arps.tile([33, TQ], f32, tag="rb", bufs=1)
                        elist = []
                        lastt = nt - 1

                        def flush_r(last):
                            for ee, tt, vv in elist:
                                if J == 0:
                                    dst, tp = r_a[0:1, vv:], (0, 0)
                                    st_, sp_ = (tt == 0), (tt == lastt)
                                elif tt % 2 == 0:
                                    dst, tp = r_a[0:1, vv:], (0, 0)
                                    st_, sp_ = (tt == 0), (tt >= lastt - 1)
                                else:
                                    dst, tp = r_b[32:33, vv:], (0, 32)
                                    st_, sp_ = (tt == 1), (tt >= lastt - 1)
                                nc.tensor.matmul(
                                    dst, lhsT=ones, rhs=ee[:, vv:],
                                    start=st_, stop=sp_, tile_position=tp,
                                )
                            elist.clear()

                        for t in range(nt):
                            # causal trapezoid: within quarter J, tile t only
                            # has valid scores for q >= 128*(t-4J); shrink the
                            # streamed width accordingly and mask only the
                            # 128-col triangle block
                            vo = max(0, P * (t - 4 * J))
                            vsl = slice(vo, TQ)
                            st_ps = stps.tile([P, TQ], f32, tag="st")
                            nc.tensor.matmul(
                                st_ps[:, vsl],
                                lhsT=KT[:, hh, P * t : P * t + P],
                                rhs=QT[:, hh, TQ * J + vo : TQ * J + TQ],
                                start=True, stop=True,
                            )
                            f = fp.tile([P, TQ], bf, tag="f")
                            nc.scalar.activation(
                                out=f[:, vsl], in_=st_ps[:, vsl], func=AF.Exp,
                                scale=SCALE,
                            )
                            if t >= 4 * J:
                                nc.vector.tensor_mul(
                                    out=f[:, vo : vo + P], in0=f[:, vo : vo + P],
                                    in1=masks[:, 384 : 384 + P],
                                )
                            e = ep.tile([P, TQ], bf, tag="e")
                            nc.vector.tensor_mul(
                                out=e[:, vsl], in0=f[:, vsl], in1=f[:, vsl]
                            )
                            elist.append((e, t, vo))
                            nc.tensor.matmul(
                                o_ps[:, vsl],
                                lhsT=V[:, t, P * hh : P * hh + P],
                                rhs=f[:, vsl],
                                start=(t == 0), stop=(t == nt - 1),
                            )
                            if len(elist) == 4 or t == nt - 1:
                                flush_r(last=(t == nt - 1))
                        # stash unnormalized OT and the r strips; normalize
                        # later.  DVE lanes cannot cross partitions, so land
                        # each strip at its own lane and DMA into rrows.
                        nc.vector.tensor_copy(out=attnT[:, hh, jsl], in_=o_ps)
                        idx = 4 * hh + J
                        rrow_a = fp.tile([1, TQ], f32, tag="rrowa", bufs=2)
                        nc.vector.tensor_copy(out=rrow_a[:], in_=r_a[:])
                        nc.sync.dma_start(
                            out=rrows[idx : idx + 1, :], in_=rrow_a[:]
                        )
                        if J > 0:
                            rrow_b = fp.tile([33, TQ], f32, tag="rrowb", bufs=2)
                            nc.vector.tensor_copy(
                                out=rrow_b[32:33, :], in_=r_b[32:33, :]
                            )
                            nc.sync.dma_start(
                                out=rrows2[idx : idx + 1, :], in_=rrow_b[32:33, :]
                            )

                # batched rsqrt of all 16 r rows: cr = exp(-0.5*ln(r))
                nc.vector.tensor_add(out=rrows[:], in0=rrows[:], in1=rrows2[:])
                lnr = attn_pool.tile([16, TQ], f32)
                nc.scalar.activation(out=lnr[:], in_=rrows[:], func=AF.Ln)
                cr16 = attn_pool.tile([16, TQ], bf)
                nc.scalar.activation(out=cr16[:], in_=lnr[:], func=AF.Exp, scale=-0.5)
                crdd = crd.tile([16, TQ], bf)
                nc.sync.dma_start(out=crdd, in_=cr16[:])
                # scale J-outer so phase D (which consumes token blocks in
                # order) unblocks after the first quarter's scales
                for J in range(NQ):
                    jsl = slice(TQ * J, TQ * J + TQ)
                    for hh in range(HL):
                        idx = 4 * hh + J
                        bap = _b.AP(
                            tensor=crdd.tensor, offset=crdd.offset + idx * TQ,
                            ap=[[0, P], [1, TQ]],
                        )
                        cbt = cbp.tile([P, TQ], bf, tag="cb")
                        nc.sync.dma_start(out=cbt[:], in_=bap)
                        nc.vector.tensor_mul(
                            out=attnT[:, hh, jsl], in0=attnT[:, hh, jsl], in1=cbt[:]
                        )

            # ---------------- Phase D: out projection (token-major) ----------------
            with (
                tc.tile_pool(name="obuf", bufs=4) as obuf,
                tc.tile_pool(name="ops3", bufs=4, space="PSUM") as ops3,
            ):
                for g4 in range(0, NT, 4):
                    pos = {}
                    for i in range(g4, g4 + 4):
                        for n in range(2):
                            po = ops3.tile(
                                [P, 512], f32, tag="po", name=f"po_{i}_{n}", bufs=8
                            )
                            pos[(i, n)] = po
                            for c in range(HL):
                                nc.tensor.matmul(
                                    po,
                                    lhsT=gT[:, c, P * i : P * i + P],
                                    rhs=wos[:, c, 512 * n : 512 * n + 512],
                                    start=(c == 0), stop=False,
                                )
                    for i in range(g4, g4 + 4):
                        for n in range(2):
                            po = pos[(i, n)]
                            for c in range(HL, KC):
                                nc.tensor.matmul(
                                    po,
                                    lhsT=attnT[:, c - HL, P * i : P * i + P],
                                    rhs=wos[:, c, 512 * n : 512 * n + 512],
                                    start=False, stop=(c == KC - 1),
                                )
                            ot = obuf.tile([P, 512], bf, tag="ot")
                            nc.scalar.copy(out=ot[:], in_=po)
                            nc.sync.dma_start(
                                out=out[P * i : P * i + P, 512 * n : 512 * n + 512],
                                in_=ot[:],
                            )
            wop.release()
            attn_pool.release()

    _split_multi_waits(nc, mybir)
    return nc


def _core_inputs(inputs, core):
    """Build the per-core input map (numpy, host-side sharding/layout)."""
    x = np.asarray(inputs["x"], dtype=np.float32)
    gamma = np.asarray(inputs["gamma"], dtype=np.float32)
    beta = np.asarray(inputs["beta"], dtype=np.float32)
    w_qkv = np.asarray(inputs["w_qkv"], dtype=np.float32)
    w_out = np.asarray(inputs["w_out"], dtype=np.float32)
    import ml_dtypes

    bf = ml_dtypes.bfloat16

    b, j = core // 2, core % 2
    sl = slice(512 * j, 512 * j + 512)
    xT = np.ascontiguousarray(x[b].T.astype(bf))
    # fold gamma into the weights (w rows are output dims; transpose first)
    wg = w_qkv * gamma[None, :]
    wqT = np.ascontiguousarray(wg[0 * D : 1 * D][sl].T.astype(bf))
    wkT = np.ascontiguousarray(wg[1 * D : 2 * D][sl].T.astype(bf))
    wvT = np.ascontiguousarray(wg[2 * D : 3 * D][sl].T.astype(bf))
    wlinT = np.ascontiguousarray(wg[3 * D : 4 * D][sl].T.astype(bf))
    wpreT = np.ascontiguousarray(wg[4 * D : 5 * D][sl].T.astype(bf))
    cols = np.r_[512 * j : 512 * j + 512, D + 512 * j : D + 512 * j + 512]
    woT = np.ascontiguousarray(w_out[:, cols].T.astype(bf))
    # beta corrections: c_* = beta @ W_slice.T (per out-dim), dim-major [128, m]
    cq = (beta @ w_qkv[0 * D : 1 * D][sl].T).astype(np.float32)
    ck = (beta @ w_qkv[1 * D : 2 * D][sl].T).astype(np.float32)
    cv = (beta @ w_qkv[2 * D : 3 * D][sl].T).astype(np.float32)
    cl = (beta @ w_qkv[3 * D : 4 * D][sl].T).astype(np.float32)
    cp = (beta @ w_qkv[4 * D : 5 * D][sl].T).astype(np.float32)
    cqkl = np.stack(
        [c.reshape(HL, P).T for c in (cq, ck, cl)], axis=1
    ).reshape(P, 3 * HL)
    cpre = np.ascontiguousarray(cp.reshape(HL, P).T)
    cvrow = cv.reshape(1, 512).astype(bf)
    # transposed causal mask template: maskT[kk, c] = 1 iff c >= kk + 384
    kk = np.arange(P)[:, None]
    cc = np.arange(896)[None, :]
    maskT = (cc >= kk + 384).astype(bf)
    return {
        "xT": xT,
        "wqT": wqT,
        "wkT": wkT,
        "wvT": wvT,
        "wlinT": wlinT,
        "wpreT": wpreT,
        "woT": woT,
        "maskT": maskT,
        "cqkl": cqkl,
        "cpre": cpre,
        "cvrow": cvrow,
    }


def _run(inputs, trace=False, trace_kwargs=None):
    from concourse.bass_utils import run_bass_kernel_spmd

    beta = np.asarray(inputs["beta"], dtype=np.float32)
    with_beta = bool(np.any(beta != 0.0))
    key = ("nc", with_beta)
    if key not in _CACHED:
        _CACHED[key] = _build_nc(with_beta=with_beta)
    nc = _CACHED[key]
    in_maps = [_core_inputs(inputs, c) for c in range(8)]
    res = run_bass_kernel_spmd(
        nc, in_maps, core_ids=list(range(8)), trace=trace,
        **(trace_kwargs or {}),
    )
    x = np.asarray(inputs["x"], dtype=np.float32)
    out = np.empty((B, L, D), dtype=np.float32)
    for b in range(B):
        out[b] = (
            x[b]
            + res.results[2 * b]["out"].astype(np.float32)
            + res.results[2 * b + 1]["out"].astype(np.float32)
        )
    return out, res


def kernel(**inputs) -> np.ndarray:
    out, _ = _run(inputs, trace=False)
    return out
